# revision 11
# baseline (speedup 1.0000x reference)
"""Trainium2 Bass kernel for the MLPConstructor2 adjacency problem.

Computes, per batch b (one NeuronCore each, 8-way data parallel over B):
    adj[i, j] = tanh(relu(x1_i @ w1 + x2_j @ w2 + b))
for the four (spatial/temporal) quadrants of a (2560, 2560) output.

v9 design (three-engine split, targeting the ~40us HBM-store floor):
- The output is 10 "units" of 256 strided rows x 2560 cols (fp16 stores,
  13.1 MB/core). 8 units run on ScalarE (fused relu-add on DVE/Pool, one
  big tanh on ACT); 2 units are computed entirely on VectorE with a
  clamped shifted-quintic tanh approximation (max err ~0.011 < 2e-2
  tolerance) that runs in fp16 4x mode: clip(x*(c0+c1*v+c2*v^2), 0, cap)
  with v=(x+g)^2 via a pow tensor_scalar.
- relu-adds for the ACT units are split between DVE (4 units) and the
  otherwise-idle GpSimd engine (4 units + both row-stat reductions).
- Column vectors are built on-chip: 32x32 VectorE transposes put x^T on
  partitions; the transposed col weights arrive via one strided
  broadcast DMA (no wp->transpose dependency); rank-32 TensorE matmuls
  run in fp16 (1 cycle/row) into fp32 PSUM; VectorE casts
  land fp16 cols in SBUF.
- Prologue: all input DMAs issue first on the sync+scalar queues, the
  ACT table load warms behind them, first tanh targets ~12us.
- First and last units split per-half for store ramp/tail.
"""

import numpy as np
from contextlib import ExitStack

import concourse.bacc as bacc
import concourse.mybir as mybir
import concourse.tile as tile
from concourse.bass_utils import run_bass_kernel_spmd

B, N, T, D = 8, 2048, 512, 32
W = N + T                     # 2560
NT, TT = N // 128, T // 128   # 16, 4 stat slots per partition
F32 = mybir.dt.float32
F32R = mybir.dt.float32r
F16 = mybir.dt.float16
QUADS = ("ss", "tt", "st", "ts")

# clamped shifted-quintic tanh(relu(x)) fit on [0, 8] (max err 0.0102;
# ~0.011 with fp16 intermediate rounding):
#   out = clip(x*(PC0 + PC1*v + PC2*v^2), 0, PCAP), v = (x + PG)^2
PG = 0.639176
PC0 = 1.076125
PC1 = -0.133401
PC2 = 0.006802
PCAP = 0.989849

# unit assignment: 8 spatial units (t=0..7), 2 temporal (t=0..1).
POLY_UNITS = "S4 + S5h0"      # computed on VectorE (1.5 units)
DVE_PRE = "all"               # GpSimd tensor ops are ~15ns/elem on HW: unusable


def _emit(tc, sp, tm, wp_in, adj):
    nc = tc.nc
    AF = mybir.ActivationFunctionType
    OP = mybir.AluOpType
    with ExitStack() as ctx:
        ctx.enter_context(nc.allow_low_precision(
            reason="fp16 intermediates; tolerance is 2e-2, fp16 adds ~1e-3"
        ))
        const = ctx.enter_context(tc.tile_pool(name="const", bufs=1))
        tmpp = ctx.enter_context(tc.tile_pool(name="tmpp", bufs=6))
        outp = ctx.enter_context(tc.tile_pool(name="outp", bufs=4))
        poly = ctx.enter_context(tc.tile_pool(name="poly", bufs=1))
        psum = ctx.enter_context(tc.tile_pool(name="psum", bufs=1, space="PSUM"))

        # ---- phase 0: every input DMA issues first ------------------------
        # wpack = [w_ss | w_tt | w_st | w_ts | b_ss b_tt b_st b_ts] (260 f32)
        wp = const.tile([128, 260], F32)
        nc.sync.dma_start(wp[:], wp_in.unsqueeze(0).broadcast_to((128, 260)))
        # stage inputs, (p t) layout: row p*nt+t at [p, t*D:(t+1)*D]
        x_sp = const.tile([128, NT * D], F32)
        nc.sync.dma_start(x_sp[:], sp.rearrange("(p t) d -> p t d", p=128))
        x_tm = const.tile([128, TT * D], F32)
        nc.sync.dma_start(x_tm[:], tm.rearrange("(p t) d -> p t d", p=128))
        # row-rotated copies (partition p <- row (64+p)*nt+t, p<64): the PE
        # column builder needs every 32-row group at partition base 0/32.
        x_sp_sh = const.tile([64, NT * D], F32)
        nc.gpsimd.dma_start(
            x_sp_sh[:], sp.rearrange("(p t) d -> p t d", p=128)[64:128]
        )
        x_tm_sh = const.tile([64, TT * D], F32)
        nc.gpsimd.dma_start(
            x_tm_sh[:], tm.rearrange("(p t) d -> p t d", p=128)[64:128]
        )

        # ---- warmup: ACT table load (behind the DMA issues) ---------------
        warm = const.tile([128, 1], F32)
        nc.vector.memset(warm[:], 0.0)
        nc.scalar.activation(warm[:], warm[:], AF.Tanh)

        # ---- DVE front: wT transpose + wmat broadcast + x^T transposes ----
        # wT[32a+d, j] = w_col_q(d): 32x32-block transpose of the (identical
        # across partitions) wp columns; then materialized to [32, 128] rows.
        wT = const.tile([128, 4 * D], F32)
        for q in range(4):
            nc.vector.transpose(
                wT[:, 32 * q : 32 * (q + 1)], wp[:, 64 * q + D : 64 * q + 2 * D]
            )
        ones = const.tile([128, 128], F32)
        nc.vector.memset(ones[:], 1.0)
        wmat = const.tile([128, 4 * 128], F16)   # q at [128q : 128(q+1)]
        for q in range(4):
            nc.vector.tensor_scalar(
                wmat[:, 128 * q : 128 * (q + 1)], ones[:],
                wT[:, 32 * q : 32 * q + 1], None, OP.mult,
            )
        # xT[32a+d, 32t+p'] = x[(32a+p')*nt + t, d]; lo = groups 0,1 and
        # hi = groups 2,3 (from the rotated staging), all at bases 0/32.
        # staging is converted to fp16 first so the PE matmuls run at
        # 1 cycle/row (fp16 x adds ~3.5e-4 col error, negligible).
        xc_sp = const.tile([64, NT * D], F16, name="xc_sp", tag="xc_sp")
        nc.vector.tensor_copy(xc_sp[:], x_sp[0:64, :])
        xc_sph = const.tile([64, NT * D], F16, name="xc_sph", tag="xc_sph")
        nc.vector.tensor_copy(xc_sph[:], x_sp_sh[:])
        xc_tm = const.tile([64, TT * D], F16, name="xc_tm", tag="xc_tm")
        nc.vector.tensor_copy(xc_tm[:], x_tm[0:64, :])
        xc_tmh = const.tile([64, TT * D], F16, name="xc_tmh", tag="xc_tmh")
        nc.vector.tensor_copy(xc_tmh[:], x_tm_sh[:])
        xT_sp = const.tile([64, NT * D], F16, name="xT_sp", tag="xT_sp")
        nc.vector.transpose(xT_sp[:], xc_sp[:])
        xT_sph = const.tile([64, NT * D], F16, name="xT_sph", tag="xT_sph")
        nc.vector.transpose(xT_sph[:], xc_sph[:])
        xT_tm = const.tile([64, TT * D], F16, name="xT_tm", tag="xT_tm")
        nc.vector.transpose(xT_tm[:], xc_tm[:])
        xT_tmh = const.tile([64, TT * D], F16, name="xT_tmh", tag="xT_tmh")
        nc.vector.transpose(xT_tmh[:], xc_tmh[:])

        def w_row(q):  # first half of w_q: row-side weights
            return wp[:, 64 * q : 64 * q + D]

        def b_q(q):
            return wp[:, 256 + q : 257 + q]

        # col[q, j] = w_col . x_j + (bias folded into row stats):
        # per 32-row group a, psum[:, ca + p'*nt + t] = sum_d wmat[d, :] *
        # xT[32a+d, (p', t)] -- f32r matmuls run 1 cycle/row at >=256 free.
        # psum column f = 512a + 32t + p'; output col j = (32a+p')*nt + t,
        # so the psum->SBUF copy un-interleaves via its (strided) input AP.
        def col_half(q_sp, q_tm, name, cast_on_act=False):
            col = const.tile([128, W], F16, name=f"col_{name}", tag=f"col_{name}")
            psn = psum.tile([128, N], F32, name=f"psn_{name}", tag="psn")
            for a in range(4):
                xs = (xT_sp, xT_sph)[a // 2]
                b = 32 * (a % 2)
                nc.tensor.matmul(
                    psn[:, 512 * a : 512 * (a + 1)],
                    wmat[b : b + 32, 128 * q_sp : 128 * (q_sp + 1)],
                    xs[b : b + 32, :],
                )

            def cast(dst, src):
                if cast_on_act:   # ScalarE is idle during the ramp
                    nc.scalar.activation(dst, src, AF.Copy)
                else:
                    nc.vector.tensor_copy(dst, src)

            for c in range(2):   # copy group-pairs as their matmuls finish
                cast(
                    col[:, 1024 * c : 1024 * (c + 1)].rearrange(
                        "q (a p t) -> q a p t", a=2, t=NT
                    ),
                    psn[:, 1024 * c : 1024 * (c + 1)].rearrange(
                        "q (a t p) -> q a p t", a=2, t=NT
                    ),
                )
            # T-part groups each get their own PSUM bank: matmul outputs
            # packed at sub-bank offsets crash the device at runtime.
            pst = psum.tile([128, N], F32, name=f"pst_{name}", tag="pst")
            for a in range(4):
                xs = (xT_tm, xT_tmh)[a // 2]
                b = 32 * (a % 2)
                nc.tensor.matmul(
                    pst[:, 512 * a : 512 * a + 128],
                    wmat[b : b + 32, 128 * q_tm : 128 * (q_tm + 1)],
                    xs[b : b + 32, :],
                )
            cast(
                col[:, N:W].rearrange("q (a p t) -> q a p t", a=4, t=TT),
                pst[:].rearrange("q (a t p) -> q a t p", a=4, t=NT)[:, :, 0:TT, :]
                .rearrange("q a t p -> q a p t"),
            )
            return col

        # row stats (slot t = row p*nt + t), quadrant biases folded in.
        def rstat(x, nt, q, dst, name):
            prod = const.tile([128, nt * D], F32, name=f"prod_{name}", tag="prod")
            x3 = x[:].rearrange("p (t d) -> p t d", t=nt)
            p3 = prod[:].rearrange("p (t d) -> p t d", t=nt)
            w3 = w_row(q).unsqueeze(1).broadcast_to((128, nt, D))
            nc.vector.tensor_tensor(p3, x3, w3, OP.mult)
            nc.vector.tensor_reduce(dst, p3, axis=mybir.AxisListType.X, op=OP.add)
            nc.vector.tensor_scalar_add(dst, dst, b_q(q))

        col_sp = col_half(0, 2, "sp", cast_on_act=True)  # w_ss2 (N) | w_st2 (T)
        r_sp = const.tile([128, 2 * NT], F32)
        rstat(x_sp, NT, 0, r_sp[:, 0:NT], "r_ss")
        rstat(x_sp, NT, 2, r_sp[:, NT:], "r_st")

        # ---- per-unit emitters -------------------------------------------
        # spatial unit t (t=0..7): rows {16p+t} (h=0) and {16p+t+8} (h=1)
        # temporal unit t (t=0..1): rows 2048 + {4p+t} and 2048 + {4p+t+2}
        def pre_add(eng, dst, col, rst, nt, t, hs):
            """dst[:, h*W + region] = max(col[region] + r_region[slot], 0)"""
            for h in range(2):
                o = h * W
                s = t + h * hs
                eng.tensor_scalar(
                    dst[:, o : o + N], col[:, 0:N],
                    rst[:, s : s + 1], 0.0, OP.add, OP.max,
                )
                eng.tensor_scalar(
                    dst[:, o + N : o + W], col[:, N:W],
                    rst[:, nt + s : nt + s + 1], 0.0, OP.add, OP.max,
                )

        def store_unit(k, ot, base, nt, t, hs, split):
            quad = adj[base : base + 128 * nt, :]
            if split:
                for h in range(2):
                    s = t + h * hs
                    nc.sync.dma_start(
                        quad.rearrange("(p r) w -> p r w", p=128)[:, s : s + 1, :],
                        ot[:, h * W : (h + 1) * W].rearrange(
                            "p (r w) -> p r w", r=1
                        ),
                    )
            else:
                # partition p -> rows base + nt*p + t and base + nt*p + t + hs
                nc.sync.dma_start(
                    quad.rearrange("(p g r) w -> p g r w", p=128, g=2)[
                        :, :, t : t + 1, :
                    ],
                    ot[:].rearrange("p (g w) -> p g w", g=2).unsqueeze(2),
                )

        def act_unit(k, t, base, nt, col, rst, hs, pre_eng, split=False):
            tmp = tmpp.tile([128, 2 * W], F16, name=f"tmp{k}", tag="tmp")
            ot = outp.tile([128, 2 * W], F16, name=f"ot{k}", tag="ot")
            pre_add(pre_eng, tmp, col, rst, nt, t, hs)
            if split:
                for h in range(2):
                    nc.scalar.activation(
                        ot[:, h * W : (h + 1) * W],
                        tmp[:, h * W : (h + 1) * W], AF.Tanh,
                    )
            else:
                nc.scalar.activation(ot[:], tmp[:], AF.Tanh)
            store_unit(k, ot, base, nt, t, hs, split)

        # shifted pre-add: y = max(s + PG, PG) = relu(s) + PG, from row
        # stats pre-shifted by PG (rsh tile). x is recovered with one ts.
        def pre_add_sh(dst, col, rsh, nt, t, hs, h_list):
            for h in h_list:
                o = h * W
                s = t + h * hs
                nc.vector.tensor_scalar(
                    dst[:, o : o + N], col[:, 0:N],
                    rsh[:, s : s + 1], PG, OP.add, OP.max,
                )
                nc.vector.tensor_scalar(
                    dst[:, o + N : o + W], col[:, N:W],
                    rsh[:, nt + s : nt + s + 1], PG, OP.add, OP.max,
                )

        def poly_half(k, t, base, nt, col, rsh, hs, ot, h_list):
            # out = clip(x*(PC0 + PC1*u + PC2*u^2), 0, PCAP), u = (x+PG)^2
            lo = min(h_list) * W
            hi = (max(h_list) + 1) * W
            y = poly.tile([128, 2 * W], F16, name=f"py{k}", tag="py")
            x = poly.tile([128, 2 * W], F16, name=f"px{k}", tag="px")
            u = poly.tile([128, 2 * W], F16, name=f"pu{k}", tag="pu")
            t1 = poly.tile([128, 2 * W], F16, name=f"pt1_{k}", tag="pt1")
            t2 = poly.tile([128, 2 * W], F16, name=f"pt2_{k}", tag="pt2")
            pre_add_sh(y, col, rsh, nt, t, hs, h_list)
            yv, xv = y[:, lo:hi], x[:, lo:hi]
            uv, t1v, t2v = u[:, lo:hi], t1[:, lo:hi], t2[:, lo:hi]
            nc.vector.tensor_scalar(xv, yv, -PG, None, OP.add)
            nc.vector.tensor_tensor(uv, yv, yv, OP.mult)
            nc.vector.tensor_scalar(t1v, uv, PC2, PC1, OP.mult, OP.add)
            nc.vector.tensor_tensor(t2v, t1v, uv, OP.mult)
            nc.vector.tensor_scalar(t1v, t2v, PC0, None, OP.add)
            nc.vector.tensor_tensor(t2v, t1v, xv, OP.mult)
            nc.vector.tensor_scalar(ot[:, lo:hi], t2v, PCAP, 0.0, OP.min, OP.max)

        # shifted row stats for the poly units (y = relu(s) + PG trick)
        rp_sp = const.tile([128, 2 * NT], F32)
        nc.vector.tensor_scalar(rp_sp[:], r_sp[:], PG, None, OP.add)

        # ---- unit schedule ------------------------------------------------
        # ACT order: casts, S0h0 S0h1 S1 S2 S3 S5 S6 S7 T0 T1h0 T1h1.
        # DVE order: front, pre0, pre1, tm-casts, pre2, pre3, pre8, pre9,
        # pre5, pre6, pre7, poly_S4 — every relu-add lands before the poly
        # chain so neither ScalarE nor the store queue ever waits on it.
        # Store order: S0h0 S0h1 S1 S2 S3 S4 S5 S6 S7 T0 T1 (completion
        # order; S4 from the poly finishes ~40us, right in its slot).
        act_unit(0, 0, 0, NT, col_sp, r_sp, NT // 2, nc.vector, split=True)
        pre1 = tmpp.tile([128, 2 * W], F16, name="tmp1", tag="tmp")
        pre_add(nc.vector, pre1, col_sp, r_sp, NT, 1, NT // 2)

        col_tm = col_half(3, 1, "tm")    # cols: w_ts2 (N) | w_tt2 (T)
        r_tm = const.tile([128, 2 * TT], F32)
        rstat(x_tm, TT, 3, r_tm[:, 0:TT], "r_ts")
        rstat(x_tm, TT, 1, r_tm[:, TT:], "r_tt")

        # S1 (tmp already built above)
        ot1 = outp.tile([128, 2 * W], F16, name="ot1", tag="ot")
        nc.scalar.activation(ot1[:], pre1[:], AF.Tanh)
        store_unit(1, ot1, 0, NT, 1, NT // 2, split=False)

        act_unit(2, 2, 0, NT, col_sp, r_sp, NT // 2, nc.vector)
        act_unit(3, 3, 0, NT, col_sp, r_sp, NT // 2, nc.vector)

        # pre-build every remaining tmp on DVE before the poly chain
        tmp8 = tmpp.tile([128, 2 * W], F16, name="tmp8", tag="tmp")
        pre_add(nc.vector, tmp8, col_tm, r_tm, TT, 0, TT // 2)
        tmp9 = tmpp.tile([128, 2 * W], F16, name="tmp9", tag="tmp")
        pre_add(nc.vector, tmp9, col_tm, r_tm, TT, 1, TT // 2)
        tmp5 = tmpp.tile([128, 2 * W], F16, name="tmp5", tag="tmp")
        pre_add(nc.vector, tmp5, col_sp, r_sp, NT, 5, NT // 2)
        tmp6 = tmpp.tile([128, 2 * W], F16, name="tmp6", tag="tmp")
        pre_add(nc.vector, tmp6, col_sp, r_sp, NT, 6, NT // 2)
        tmp7 = tmpp.tile([128, 2 * W], F16, name="tmp7", tag="tmp")
        pre_add(nc.vector, tmp7, col_sp, r_sp, NT, 7, NT // 2)

        # poly S4 (both halves) on DVE; store slots between S3 and S5
        ot4 = outp.tile([128, 2 * W], F16, name="ot4", tag="ot")
        poly_half(4, 4, 0, NT, col_sp, rp_sp, NT // 2, ot4, [0, 1])
        store_unit(4, ot4, 0, NT, 4, NT // 2, split=False)

        for k, tmpk, t, base, nt in (
            (5, tmp5, 5, 0, NT), (6, tmp6, 6, 0, NT), (7, tmp7, 7, 0, NT),
            (8, tmp8, 0, N, TT),
        ):
            otk = outp.tile([128, 2 * W], F16, name=f"ot{k}", tag="ot")
            nc.scalar.activation(otk[:], tmpk[:], AF.Tanh)
            store_unit(k, otk, base, nt, t, nt // 2, split=False)

        ot9 = outp.tile([128, 2 * W], F16, name="ot9", tag="ot")
        for h in range(2):
            nc.scalar.activation(
                ot9[:, h * W : (h + 1) * W], tmp9[:, h * W : (h + 1) * W],
                AF.Tanh,
            )
        store_unit(9, ot9, N, TT, 1, TT // 2, split=True)

def build_nc(num_devices=8):
    nc = bacc.Bacc(
        "TRN2",
        target_bir_lowering=False,
        debug=False,
        enable_asserts=True,
        num_devices=num_devices,
    )
    sp = nc.dram_tensor("spatial_nodes", (N, D), F32, kind="ExternalInput").ap()
    tm = nc.dram_tensor("temporal_nodes", (T, D), F32, kind="ExternalInput").ap()
    wp = nc.dram_tensor("wpack", (260,), F32, kind="ExternalInput").ap()
    adj = nc.dram_tensor("adj", (W, W), F16, kind="ExternalOutput").ap()

    with tile.TileContext(nc) as tc:
        _emit(tc, sp, tm, wp, adj)
    nc.compile()
    return nc


def make_in_maps(inputs):
    wpack = np.concatenate(
        [np.asarray(inputs[f"w_{nm}"], np.float32).reshape(-1) for nm in QUADS]
        + [np.asarray(inputs[f"b_{nm}"], np.float32).reshape(-1) for nm in QUADS]
    )
    in_maps = []
    for b in range(B):
        m = {
            "spatial_nodes": np.ascontiguousarray(inputs["spatial_nodes"][b], np.float32),
            "temporal_nodes": np.ascontiguousarray(inputs["temporal_nodes"][b], np.float32),
            "wpack": wpack,
        }
        in_maps.append(m)
    return in_maps


_NC = {}


def run(inputs, trace=False, trace_cores=None):
    if 8 not in _NC:
        _NC[8] = build_nc(8)
    res = run_bass_kernel_spmd(
        _NC[8], make_in_maps(inputs), core_ids=list(range(B)), trace=trace,
        trace_cores=trace_cores,
    )
    out = np.stack(
        [res.results[i]["adj"].astype(np.float32) for i in range(B)], axis=0
    )
    return out, res


def kernel(**inputs) -> np.ndarray:
    out, _ = run(inputs, trace=False)
    return out


# revision 12
# speedup vs baseline: 1.1230x; 1.1230x over previous
"""Trainium2 Bass kernel for the MLPConstructor2 adjacency problem.

Computes, per batch b (one NeuronCore each, 8-way data parallel over B):
    adj[i, j] = tanh(relu(x1_i @ w1 + x2_j @ w2 + b))
for the four (spatial/temporal) quadrants of a (2560, 2560) output.

v9 design (three-engine split, targeting the ~40us HBM-store floor):
- The output is 10 "units" of 256 strided rows x 2560 cols (fp16 stores,
  13.1 MB/core). 8 units run on ScalarE (fused relu-add on DVE/Pool, one
  big tanh on ACT); 2 units are computed entirely on VectorE with a
  clamped shifted-quintic tanh approximation (max err ~0.011 < 2e-2
  tolerance) that runs in fp16 4x mode: clip(x*(c0+c1*v+c2*v^2), 0, cap)
  with v=(x+g)^2 via a pow tensor_scalar.
- relu-adds for the ACT units are split between DVE (4 units) and the
  otherwise-idle GpSimd engine (4 units + both row-stat reductions).
- Column vectors are built on-chip: 32x32 VectorE transposes put x^T on
  partitions; the transposed col weights arrive via one strided
  broadcast DMA (no wp->transpose dependency); rank-32 TensorE matmuls
  run in fp16 (1 cycle/row) into fp32 PSUM; VectorE casts
  land fp16 cols in SBUF.
- Prologue: all input DMAs issue first on the sync+scalar queues, the
  ACT table load warms behind them, first tanh targets ~12us.
- First and last units split per-half for store ramp/tail.
"""

import numpy as np
from contextlib import ExitStack

import concourse.bacc as bacc
import concourse.mybir as mybir
import concourse.tile as tile
from concourse.bass_utils import run_bass_kernel_spmd

B, N, T, D = 8, 2048, 512, 32
W = N + T                     # 2560
NT, TT = N // 128, T // 128   # 16, 4 stat slots per partition
F32 = mybir.dt.float32
F32R = mybir.dt.float32r
F16 = mybir.dt.float16
QUADS = ("ss", "tt", "st", "ts")

# clamped shifted-quintic tanh(relu(x)) fit on [0, 8] (max err 0.0102;
# ~0.011 with fp16 intermediate rounding):
#   out = clip(x*(PC0 + PC1*v + PC2*v^2), 0, PCAP), v = (x + PG)^2
PG = 0.639176
PC0 = 1.076125
PC1 = -0.133401
PC2 = 0.006802
PCAP = 0.989849

# unit assignment: 8 spatial units (t=0..7), 2 temporal (t=0..1).
POLY_UNITS = "S4 + S5h0"      # computed on VectorE (1.5 units)
DVE_PRE = "all"               # GpSimd tensor ops are ~15ns/elem on HW: unusable


def _emit(tc, sp, tm, wp_in, adj):
    nc = tc.nc
    AF = mybir.ActivationFunctionType
    OP = mybir.AluOpType
    with ExitStack() as ctx:
        ctx.enter_context(nc.allow_low_precision(
            reason="fp16 intermediates; tolerance is 2e-2, fp16 adds ~1e-3"
        ))
        const = ctx.enter_context(tc.tile_pool(name="const", bufs=1))
        tmpp = ctx.enter_context(tc.tile_pool(name="tmpp", bufs=6))
        outp = ctx.enter_context(tc.tile_pool(name="outp", bufs=4))
        poly = ctx.enter_context(tc.tile_pool(name="poly", bufs=1))
        pout = ctx.enter_context(tc.tile_pool(name="pout", bufs=1))
        psum = ctx.enter_context(tc.tile_pool(name="psum", bufs=1, space="PSUM"))

        # ---- phase 0: every input DMA issues first ------------------------
        # wpack = [w_ss | w_tt | w_st | w_ts | b_ss b_tt b_st b_ts] (260 f32)
        wp = const.tile([128, 260], F32)
        nc.sync.dma_start(wp[:], wp_in.unsqueeze(0).broadcast_to((128, 260)))
        # stage inputs, (p t) layout: row p*nt+t at [p, t*D:(t+1)*D]
        x_sp = const.tile([128, NT * D], F32)
        nc.sync.dma_start(x_sp[:], sp.rearrange("(p t) d -> p t d", p=128))
        x_tm = const.tile([128, TT * D], F32)
        nc.sync.dma_start(x_tm[:], tm.rearrange("(p t) d -> p t d", p=128))
        # row-rotated copies (partition p <- row (64+p)*nt+t, p<64): the PE
        # column builder needs every 32-row group at partition base 0/32.
        x_sp_sh = const.tile([64, NT * D], F32)
        nc.gpsimd.dma_start(
            x_sp_sh[:], sp.rearrange("(p t) d -> p t d", p=128)[64:128]
        )
        x_tm_sh = const.tile([64, TT * D], F32)
        nc.gpsimd.dma_start(
            x_tm_sh[:], tm.rearrange("(p t) d -> p t d", p=128)[64:128]
        )

        # ---- warmup: ACT table load (behind the DMA issues) ---------------
        warm = const.tile([128, 1], F32)
        nc.vector.memset(warm[:], 0.0)
        nc.scalar.activation(warm[:], warm[:], AF.Tanh)

        # ---- DVE front: wT transpose + wmat broadcast + x^T transposes ----
        # wT[32a+d, j] = w_col_q(d): 32x32-block transpose of the (identical
        # across partitions) wp columns; then materialized to [32, 128] rows.
        wT = const.tile([128, 4 * D], F32)
        for q in range(4):
            nc.vector.transpose(
                wT[:, 32 * q : 32 * (q + 1)], wp[:, 64 * q + D : 64 * q + 2 * D]
            )
        ones = const.tile([128, 128], F32)
        nc.vector.memset(ones[:], 1.0)
        wmat = const.tile([128, 4 * 128], F16)   # q at [128q : 128(q+1)]
        for q in range(4):
            nc.vector.tensor_scalar(
                wmat[:, 128 * q : 128 * (q + 1)], ones[:],
                wT[:, 32 * q : 32 * q + 1], None, OP.mult,
            )
        # xT[32a+d, 32t+p'] = x[(32a+p')*nt + t, d]; lo = groups 0,1 and
        # hi = groups 2,3 (from the rotated staging), all at bases 0/32.
        # staging is converted to fp16 first so the PE matmuls run at
        # 1 cycle/row (fp16 x adds ~3.5e-4 col error, negligible).
        xc_sp = const.tile([64, NT * D], F16, name="xc_sp", tag="xc_sp")
        nc.vector.tensor_copy(xc_sp[:], x_sp[0:64, :])
        xc_sph = const.tile([64, NT * D], F16, name="xc_sph", tag="xc_sph")
        nc.vector.tensor_copy(xc_sph[:], x_sp_sh[:])
        xc_tm = const.tile([64, TT * D], F16, name="xc_tm", tag="xc_tm")
        nc.vector.tensor_copy(xc_tm[:], x_tm[0:64, :])
        xc_tmh = const.tile([64, TT * D], F16, name="xc_tmh", tag="xc_tmh")
        nc.vector.tensor_copy(xc_tmh[:], x_tm_sh[:])
        xT_sp = const.tile([64, NT * D], F16, name="xT_sp", tag="xT_sp")
        nc.vector.transpose(xT_sp[:], xc_sp[:])
        xT_sph = const.tile([64, NT * D], F16, name="xT_sph", tag="xT_sph")
        nc.vector.transpose(xT_sph[:], xc_sph[:])
        xT_tm = const.tile([64, TT * D], F16, name="xT_tm", tag="xT_tm")
        nc.vector.transpose(xT_tm[:], xc_tm[:])
        xT_tmh = const.tile([64, TT * D], F16, name="xT_tmh", tag="xT_tmh")
        nc.vector.transpose(xT_tmh[:], xc_tmh[:])

        def w_row(q):  # first half of w_q: row-side weights
            return wp[:, 64 * q : 64 * q + D]

        def b_q(q):
            return wp[:, 256 + q : 257 + q]

        # col[q, j] = w_col . x_j + (bias folded into row stats):
        # per 32-row group a, psum[:, ca + p'*nt + t] = sum_d wmat[d, :] *
        # xT[32a+d, (p', t)] -- f32r matmuls run 1 cycle/row at >=256 free.
        # psum column f = 512a + 32t + p'; output col j = (32a+p')*nt + t,
        # so the psum->SBUF copy un-interleaves via its (strided) input AP.
        def col_half(q_sp, q_tm, name, cast_on_act=False):
            col = const.tile([128, W], F16, name=f"col_{name}", tag=f"col_{name}")
            psn = psum.tile([128, N], F32, name=f"psn_{name}", tag="psn")
            for a in range(4):
                xs = (xT_sp, xT_sph)[a // 2]
                b = 32 * (a % 2)
                nc.tensor.matmul(
                    psn[:, 512 * a : 512 * (a + 1)],
                    wmat[b : b + 32, 128 * q_sp : 128 * (q_sp + 1)],
                    xs[b : b + 32, :],
                )

            def cast(dst, src):
                if cast_on_act:   # ScalarE is idle during the ramp
                    nc.scalar.activation(dst, src, AF.Copy)
                else:
                    nc.vector.tensor_copy(dst, src)

            for c in range(2):   # copy group-pairs as their matmuls finish
                cast(
                    col[:, 1024 * c : 1024 * (c + 1)].rearrange(
                        "q (a p t) -> q a p t", a=2, t=NT
                    ),
                    psn[:, 1024 * c : 1024 * (c + 1)].rearrange(
                        "q (a t p) -> q a p t", a=2, t=NT
                    ),
                )
            # T-part groups each get their own PSUM bank: matmul outputs
            # packed at sub-bank offsets crash the device at runtime.
            pst = psum.tile([128, N], F32, name=f"pst_{name}", tag="pst")
            for a in range(4):
                xs = (xT_tm, xT_tmh)[a // 2]
                b = 32 * (a % 2)
                nc.tensor.matmul(
                    pst[:, 512 * a : 512 * a + 128],
                    wmat[b : b + 32, 128 * q_tm : 128 * (q_tm + 1)],
                    xs[b : b + 32, :],
                )
            cast(
                col[:, N:W].rearrange("q (a p t) -> q a p t", a=4, t=TT),
                pst[:].rearrange("q (a t p) -> q a t p", a=4, t=NT)[:, :, 0:TT, :]
                .rearrange("q a t p -> q a p t"),
            )
            return col

        # row stats (slot t = row p*nt + t), quadrant biases folded in.
        def rstat(x, nt, q, dst, name):
            prod = const.tile([128, nt * D], F32, name=f"prod_{name}", tag="prod")
            x3 = x[:].rearrange("p (t d) -> p t d", t=nt)
            p3 = prod[:].rearrange("p (t d) -> p t d", t=nt)
            w3 = w_row(q).unsqueeze(1).broadcast_to((128, nt, D))
            nc.vector.tensor_tensor(p3, x3, w3, OP.mult)
            nc.vector.tensor_reduce(dst, p3, axis=mybir.AxisListType.X, op=OP.add)
            nc.vector.tensor_scalar_add(dst, dst, b_q(q))

        col_sp = col_half(0, 2, "sp", cast_on_act=True)  # w_ss2 (N) | w_st2 (T)
        r_sp = const.tile([128, 2 * NT], F32)
        rstat(x_sp, NT, 0, r_sp[:, 0:NT], "r_ss")
        rstat(x_sp, NT, 2, r_sp[:, NT:], "r_st")

        # ---- per-unit emitters -------------------------------------------
        # spatial unit t (t=0..7): rows {16p+t} (h=0) and {16p+t+8} (h=1)
        # temporal unit t (t=0..1): rows 2048 + {4p+t} and 2048 + {4p+t+2}
        def pre_add(eng, dst, col, rst, nt, t, hs):
            """dst[:, h*W + region] = max(col[region] + r_region[slot], 0)"""
            for h in range(2):
                o = h * W
                s = t + h * hs
                eng.tensor_scalar(
                    dst[:, o : o + N], col[:, 0:N],
                    rst[:, s : s + 1], 0.0, OP.add, OP.max,
                )
                eng.tensor_scalar(
                    dst[:, o + N : o + W], col[:, N:W],
                    rst[:, nt + s : nt + s + 1], 0.0, OP.add, OP.max,
                )

        def store_unit(k, ot, base, nt, t, hs, split):
            quad = adj[base : base + 128 * nt, :]
            if split:
                for h in range(2):
                    s = t + h * hs
                    nc.sync.dma_start(
                        quad.rearrange("(p r) w -> p r w", p=128)[:, s : s + 1, :],
                        ot[:, h * W : (h + 1) * W].rearrange(
                            "p (r w) -> p r w", r=1
                        ),
                    )
            else:
                # partition p -> rows base + nt*p + t and base + nt*p + t + hs
                nc.sync.dma_start(
                    quad.rearrange("(p g r) w -> p g r w", p=128, g=2)[
                        :, :, t : t + 1, :
                    ],
                    ot[:].rearrange("p (g w) -> p g w", g=2).unsqueeze(2),
                )

        def act_unit(k, t, base, nt, col, rst, hs, pre_eng, split=False):
            tmp = tmpp.tile([128, 2 * W], F16, name=f"tmp{k}", tag="tmp")
            ot = outp.tile([128, 2 * W], F16, name=f"ot{k}", tag="ot")
            pre_add(pre_eng, tmp, col, rst, nt, t, hs)
            if split:
                for h in range(2):
                    nc.scalar.activation(
                        ot[:, h * W : (h + 1) * W],
                        tmp[:, h * W : (h + 1) * W], AF.Tanh,
                    )
            else:
                nc.scalar.activation(ot[:], tmp[:], AF.Tanh)
            store_unit(k, ot, base, nt, t, hs, split)

        # shifted pre-add: y = max(s + PG, PG) = relu(s) + PG, from row
        # stats pre-shifted by PG (rsh tile). x is recovered with one ts.
        def pre_add_sh(dst, col, rsh, nt, t, hs, h_list):
            for h in h_list:
                o = h * W
                s = t + h * hs
                nc.vector.tensor_scalar(
                    dst[:, o : o + N], col[:, 0:N],
                    rsh[:, s : s + 1], PG, OP.add, OP.max,
                )
                nc.vector.tensor_scalar(
                    dst[:, o + N : o + W], col[:, N:W],
                    rsh[:, nt + s : nt + s + 1], PG, OP.add, OP.max,
                )

        def poly_half(k, t, base, nt, col, rsh, hs, ot, h_list):
            # out = clip(x*(PC0 + PC1*u + PC2*u^2), 0, PCAP), u = (x+PG)^2
            lo = min(h_list) * W
            hi = (max(h_list) + 1) * W
            y = poly.tile([128, 2 * W], F16, name=f"py{k}", tag="py")
            x = poly.tile([128, 2 * W], F16, name=f"px{k}", tag="px")
            u = poly.tile([128, 2 * W], F16, name=f"pu{k}", tag="pu")
            t1 = poly.tile([128, 2 * W], F16, name=f"pt1_{k}", tag="pt1")
            t2 = poly.tile([128, 2 * W], F16, name=f"pt2_{k}", tag="pt2")
            pre_add_sh(y, col, rsh, nt, t, hs, h_list)
            yv, xv = y[:, lo:hi], x[:, lo:hi]
            uv, t1v, t2v = u[:, lo:hi], t1[:, lo:hi], t2[:, lo:hi]
            nc.vector.tensor_scalar(xv, yv, -PG, None, OP.add)
            nc.vector.tensor_tensor(uv, yv, yv, OP.mult)
            nc.vector.tensor_scalar(t1v, uv, PC2, PC1, OP.mult, OP.add)
            nc.vector.tensor_tensor(t2v, t1v, uv, OP.mult)
            nc.vector.tensor_scalar(t1v, t2v, PC0, None, OP.add)
            nc.vector.tensor_tensor(t2v, t1v, xv, OP.mult)
            nc.vector.tensor_scalar(ot[:, lo:hi], t2v, PCAP, 0.0, OP.min, OP.max)

        # shifted row stats for the poly units (y = relu(s) + PG trick)
        rp_sp = const.tile([128, 2 * NT], F32)
        nc.vector.tensor_scalar(rp_sp[:], r_sp[:], PG, None, OP.add)

        # ---- unit schedule ------------------------------------------------
        # ACT order: casts, S0h0 S0h1 S1 S2 S3 S5 S6 S7 T0 T1h0 T1h1.
        # DVE order: front, pre0, pre1, tm-casts, pre2, pre3, pre8, pre9,
        # pre5, pre6, pre7, poly_S4 — every relu-add lands before the poly
        # chain so neither ScalarE nor the store queue ever waits on it.
        # Store order: S0h0 S0h1 S1 S2 S3 S4 S5 S6 S7 T0 T1 (completion
        # order; S4 from the poly finishes ~40us, right in its slot).
        act_unit(0, 0, 0, NT, col_sp, r_sp, NT // 2, nc.vector, split=True)
        pre1 = tmpp.tile([128, 2 * W], F16, name="tmp1", tag="tmp")
        pre_add(nc.vector, pre1, col_sp, r_sp, NT, 1, NT // 2)

        col_tm = col_half(3, 1, "tm")    # cols: w_ts2 (N) | w_tt2 (T)
        r_tm = const.tile([128, 2 * TT], F32)
        rstat(x_tm, TT, 3, r_tm[:, 0:TT], "r_ts")
        rstat(x_tm, TT, 1, r_tm[:, TT:], "r_tt")

        # S1 (tmp already built above)
        ot1 = outp.tile([128, 2 * W], F16, name="ot1", tag="ot")
        nc.scalar.activation(ot1[:], pre1[:], AF.Tanh)
        store_unit(1, ot1, 0, NT, 1, NT // 2, split=False)

        act_unit(2, 2, 0, NT, col_sp, r_sp, NT // 2, nc.vector)
        act_unit(3, 3, 0, NT, col_sp, r_sp, NT // 2, nc.vector)

        # poly S4 (both halves) on DVE right after S3's pre-add, so it
        # completes ~35us and its store slots between S3's and S5's.
        ot4 = pout.tile([128, 2 * W], F16, name="ot4", tag="ot4")
        poly_half(4, 4, 0, NT, col_sp, rp_sp, NT // 2, ot4, [0, 1])
        store_unit(4, ot4, 0, NT, 4, NT // 2, split=False)

        # remaining tmps on DVE after the poly chain (ACT reaches S5 at
        # ~36us; DVE delivers it right then, the rest run ahead)
        tmp5 = tmpp.tile([128, 2 * W], F16, name="tmp5", tag="tmp")
        pre_add(nc.vector, tmp5, col_sp, r_sp, NT, 5, NT // 2)
        tmp6 = tmpp.tile([128, 2 * W], F16, name="tmp6", tag="tmp")
        pre_add(nc.vector, tmp6, col_sp, r_sp, NT, 6, NT // 2)
        tmp7 = tmpp.tile([128, 2 * W], F16, name="tmp7", tag="tmp")
        pre_add(nc.vector, tmp7, col_sp, r_sp, NT, 7, NT // 2)
        tmp8 = tmpp.tile([128, 2 * W], F16, name="tmp8", tag="tmp")
        pre_add(nc.vector, tmp8, col_tm, r_tm, TT, 0, TT // 2)
        tmp9 = tmpp.tile([128, 2 * W], F16, name="tmp9", tag="tmp")
        pre_add(nc.vector, tmp9, col_tm, r_tm, TT, 1, TT // 2)

        for k, tmpk, t, base, nt in (
            (5, tmp5, 5, 0, NT), (6, tmp6, 6, 0, NT), (7, tmp7, 7, 0, NT),
            (8, tmp8, 0, N, TT),
        ):
            otk = outp.tile([128, 2 * W], F16, name=f"ot{k}", tag="ot")
            nc.scalar.activation(otk[:], tmpk[:], AF.Tanh)
            store_unit(k, otk, base, nt, t, nt // 2, split=False)

        ot9 = outp.tile([128, 2 * W], F16, name="ot9", tag="ot")
        for h in range(2):
            nc.scalar.activation(
                ot9[:, h * W : (h + 1) * W], tmp9[:, h * W : (h + 1) * W],
                AF.Tanh,
            )
        store_unit(9, ot9, N, TT, 1, TT // 2, split=True)

def build_nc(num_devices=8):
    nc = bacc.Bacc(
        "TRN2",
        target_bir_lowering=False,
        debug=False,
        enable_asserts=True,
        num_devices=num_devices,
    )
    sp = nc.dram_tensor("spatial_nodes", (N, D), F32, kind="ExternalInput").ap()
    tm = nc.dram_tensor("temporal_nodes", (T, D), F32, kind="ExternalInput").ap()
    wp = nc.dram_tensor("wpack", (260,), F32, kind="ExternalInput").ap()
    adj = nc.dram_tensor("adj", (W, W), F16, kind="ExternalOutput").ap()

    with tile.TileContext(nc) as tc:
        _emit(tc, sp, tm, wp, adj)
    nc.compile()
    return nc


def make_in_maps(inputs):
    wpack = np.concatenate(
        [np.asarray(inputs[f"w_{nm}"], np.float32).reshape(-1) for nm in QUADS]
        + [np.asarray(inputs[f"b_{nm}"], np.float32).reshape(-1) for nm in QUADS]
    )
    in_maps = []
    for b in range(B):
        m = {
            "spatial_nodes": np.ascontiguousarray(inputs["spatial_nodes"][b], np.float32),
            "temporal_nodes": np.ascontiguousarray(inputs["temporal_nodes"][b], np.float32),
            "wpack": wpack,
        }
        in_maps.append(m)
    return in_maps


_NC = {}


def run(inputs, trace=False, trace_cores=None):
    if 8 not in _NC:
        _NC[8] = build_nc(8)
    res = run_bass_kernel_spmd(
        _NC[8], make_in_maps(inputs), core_ids=list(range(B)), trace=trace,
        trace_cores=trace_cores,
    )
    out = np.stack(
        [res.results[i]["adj"].astype(np.float32) for i in range(B)], axis=0
    )
    return out, res


def kernel(**inputs) -> np.ndarray:
    out, _ = run(inputs, trace=False)
    return out


# revision 13
# speedup vs baseline: 1.1499x; 1.0240x over previous
"""Trainium2 Bass kernel for the MLPConstructor2 adjacency problem.

Computes, per batch b (one NeuronCore each, 8-way data parallel over B):
    adj[i, j] = tanh(relu(x1_i @ w1 + x2_j @ w2 + b))
for the four (spatial/temporal) quadrants of a (2560, 2560) output.

v9 design (three-engine split, targeting the ~40us HBM-store floor):
- The output is 10 "units" of 256 strided rows x 2560 cols (fp16 stores,
  13.1 MB/core). 8 units run on ScalarE (fused relu-add on DVE/Pool, one
  big tanh on ACT); 2 units are computed entirely on VectorE with a
  clamped shifted-quintic tanh approximation (max err ~0.011 < 2e-2
  tolerance) that runs in fp16 4x mode: clip(x*(c0+c1*v+c2*v^2), 0, cap)
  with v=(x+g)^2 via a pow tensor_scalar.
- relu-adds for the ACT units are split between DVE (4 units) and the
  otherwise-idle GpSimd engine (4 units + both row-stat reductions).
- Column vectors are built on-chip: 32x32 VectorE transposes put x^T on
  partitions; the transposed col weights arrive via one strided
  broadcast DMA (no wp->transpose dependency); rank-32 TensorE matmuls
  run in fp16 (1 cycle/row) into fp32 PSUM; VectorE casts
  land fp16 cols in SBUF.
- Prologue: all input DMAs issue first on the sync+scalar queues, the
  ACT table load warms behind them, first tanh targets ~12us.
- First and last units split per-half for store ramp/tail.
"""

import numpy as np
from contextlib import ExitStack

import concourse.bacc as bacc
import concourse.mybir as mybir
import concourse.tile as tile
from concourse.bass_utils import run_bass_kernel_spmd

B, N, T, D = 8, 2048, 512, 32
W = N + T                     # 2560
NT, TT = N // 128, T // 128   # 16, 4 stat slots per partition
F32 = mybir.dt.float32
F32R = mybir.dt.float32r
F16 = mybir.dt.float16
QUADS = ("ss", "tt", "st", "ts")

# clamped shifted-quintic tanh(relu(x)) fit on [0, 8] (max err 0.0102;
# ~0.011 with fp16 intermediate rounding):
#   out = clip(x*(PC0 + PC1*v + PC2*v^2), 0, PCAP), v = (x + PG)^2
PG = 0.639176
PC0 = 1.076125
PC1 = -0.133401
PC2 = 0.006802
PCAP = 0.989849

# unit assignment: 8 spatial units (t=0..7), 2 temporal (t=0..1).
POLY_UNITS = "S4 + S5h0"      # computed on VectorE (1.5 units)
DVE_PRE = "all"               # GpSimd tensor ops are ~15ns/elem on HW: unusable


def _emit(tc, sp, tm, wp_in, adj):
    nc = tc.nc
    AF = mybir.ActivationFunctionType
    OP = mybir.AluOpType
    with ExitStack() as ctx:
        ctx.enter_context(nc.allow_low_precision(
            reason="fp16 intermediates; tolerance is 2e-2, fp16 adds ~1e-3"
        ))
        const = ctx.enter_context(tc.tile_pool(name="const", bufs=1))
        tmpp = ctx.enter_context(tc.tile_pool(name="tmpp", bufs=6))
        outp = ctx.enter_context(tc.tile_pool(name="outp", bufs=4))
        poly = ctx.enter_context(tc.tile_pool(name="poly", bufs=1))
        pout = ctx.enter_context(tc.tile_pool(name="pout", bufs=1))
        psum = ctx.enter_context(tc.tile_pool(name="psum", bufs=1, space="PSUM"))

        # ---- phase 0: every input DMA issues first ------------------------
        # wpack = [w_ss | w_tt | w_st | w_ts | b_ss b_tt b_st b_ts] (260 f32)
        wp = const.tile([128, 260], F32)
        nc.sync.dma_start(wp[:], wp_in.unsqueeze(0).broadcast_to((128, 260)))
        # stage inputs, (p t) layout: row p*nt+t at [p, t*D:(t+1)*D]
        x_sp = const.tile([128, NT * D], F32)
        nc.sync.dma_start(x_sp[:], sp.rearrange("(p t) d -> p t d", p=128))
        x_tm = const.tile([128, TT * D], F32)
        nc.sync.dma_start(x_tm[:], tm.rearrange("(p t) d -> p t d", p=128))
        # row-rotated copies (partition p <- row (64+p)*nt+t, p<64): the PE
        # column builder needs every 32-row group at partition base 0/32.
        x_sp_sh = const.tile([64, NT * D], F32)
        nc.gpsimd.dma_start(
            x_sp_sh[:], sp.rearrange("(p t) d -> p t d", p=128)[64:128]
        )
        x_tm_sh = const.tile([64, TT * D], F32)
        nc.gpsimd.dma_start(
            x_tm_sh[:], tm.rearrange("(p t) d -> p t d", p=128)[64:128]
        )

        # ---- warmup: ACT table load (behind the DMA issues) ---------------
        warm = const.tile([128, 1], F32)
        nc.vector.memset(warm[:], 0.0)
        nc.scalar.activation(warm[:], warm[:], AF.Tanh)

        # ---- DVE front: wT transpose + wmat broadcast + x^T transposes ----
        # wT[32a+d, j] = w_col_q(d): 32x32-block transpose of the (identical
        # across partitions) wp columns; then materialized to [32, 128] rows.
        wT = const.tile([128, 4 * D], F32)
        for q in range(4):
            nc.vector.transpose(
                wT[:, 32 * q : 32 * (q + 1)], wp[:, 64 * q + D : 64 * q + 2 * D]
            )
        ones = const.tile([128, 128], F32)
        nc.vector.memset(ones[:], 1.0)
        wmat = const.tile([128, 4 * 128], F16)   # q at [128q : 128(q+1)]
        for q in range(4):
            nc.vector.tensor_scalar(
                wmat[:, 128 * q : 128 * (q + 1)], ones[:],
                wT[:, 32 * q : 32 * q + 1], None, OP.mult,
            )
        # xT[32a+d, 32t+p'] = x[(32a+p')*nt + t, d]; lo = groups 0,1 and
        # hi = groups 2,3 (from the rotated staging), all at bases 0/32.
        # staging is converted to fp16 first so the PE matmuls run at
        # 1 cycle/row (fp16 x adds ~3.5e-4 col error, negligible).
        xc_sp = const.tile([64, NT * D], F16, name="xc_sp", tag="xc_sp")
        nc.vector.tensor_copy(xc_sp[:], x_sp[0:64, :])
        xc_sph = const.tile([64, NT * D], F16, name="xc_sph", tag="xc_sph")
        nc.vector.tensor_copy(xc_sph[:], x_sp_sh[:])
        xc_tm = const.tile([64, TT * D], F16, name="xc_tm", tag="xc_tm")
        nc.vector.tensor_copy(xc_tm[:], x_tm[0:64, :])
        xc_tmh = const.tile([64, TT * D], F16, name="xc_tmh", tag="xc_tmh")
        nc.vector.tensor_copy(xc_tmh[:], x_tm_sh[:])
        xT_sp = const.tile([64, NT * D], F16, name="xT_sp", tag="xT_sp")
        nc.vector.transpose(xT_sp[:], xc_sp[:])
        xT_sph = const.tile([64, NT * D], F16, name="xT_sph", tag="xT_sph")
        nc.vector.transpose(xT_sph[:], xc_sph[:])
        xT_tm = const.tile([64, TT * D], F16, name="xT_tm", tag="xT_tm")
        nc.vector.transpose(xT_tm[:], xc_tm[:])
        xT_tmh = const.tile([64, TT * D], F16, name="xT_tmh", tag="xT_tmh")
        nc.vector.transpose(xT_tmh[:], xc_tmh[:])

        def w_row(q):  # first half of w_q: row-side weights
            return wp[:, 64 * q : 64 * q + D]

        def b_q(q):
            return wp[:, 256 + q : 257 + q]

        # col[q, j] = w_col . x_j + (bias folded into row stats):
        # per 32-row group a, psum[:, ca + p'*nt + t] = sum_d wmat[d, :] *
        # xT[32a+d, (p', t)] -- f32r matmuls run 1 cycle/row at >=256 free.
        # psum column f = 512a + 32t + p'; output col j = (32a+p')*nt + t,
        # so the psum->SBUF copy un-interleaves via its (strided) input AP.
        def col_half(q_sp, q_tm, name, cast_on_act=False):
            col = const.tile([128, W], F16, name=f"col_{name}", tag=f"col_{name}")
            psn = psum.tile([128, N], F32, name=f"psn_{name}", tag="psn")
            for a in range(4):
                xs = (xT_sp, xT_sph)[a // 2]
                b = 32 * (a % 2)
                nc.tensor.matmul(
                    psn[:, 512 * a : 512 * (a + 1)],
                    wmat[b : b + 32, 128 * q_sp : 128 * (q_sp + 1)],
                    xs[b : b + 32, :],
                )

            def cast(dst, src):
                if cast_on_act:   # ScalarE is idle during the ramp
                    nc.scalar.activation(dst, src, AF.Copy)
                else:
                    nc.vector.tensor_copy(dst, src)

            for c in range(2):   # copy group-pairs as their matmuls finish
                cast(
                    col[:, 1024 * c : 1024 * (c + 1)].rearrange(
                        "q (a p t) -> q a p t", a=2, t=NT
                    ),
                    psn[:, 1024 * c : 1024 * (c + 1)].rearrange(
                        "q (a t p) -> q a p t", a=2, t=NT
                    ),
                )
            # T-part groups each get their own PSUM bank: matmul outputs
            # packed at sub-bank offsets crash the device at runtime.
            pst = psum.tile([128, N], F32, name=f"pst_{name}", tag="pst")
            for a in range(4):
                xs = (xT_tm, xT_tmh)[a // 2]
                b = 32 * (a % 2)
                nc.tensor.matmul(
                    pst[:, 512 * a : 512 * a + 128],
                    wmat[b : b + 32, 128 * q_tm : 128 * (q_tm + 1)],
                    xs[b : b + 32, :],
                )
            cast(
                col[:, N:W].rearrange("q (a p t) -> q a p t", a=4, t=TT),
                pst[:].rearrange("q (a t p) -> q a t p", a=4, t=NT)[:, :, 0:TT, :]
                .rearrange("q a t p -> q a p t"),
            )
            return col

        # row stats (slot t = row p*nt + t), quadrant biases folded in.
        def rstat(x, nt, q, dst, name):
            prod = const.tile([128, nt * D], F32, name=f"prod_{name}", tag="prod")
            x3 = x[:].rearrange("p (t d) -> p t d", t=nt)
            p3 = prod[:].rearrange("p (t d) -> p t d", t=nt)
            w3 = w_row(q).unsqueeze(1).broadcast_to((128, nt, D))
            nc.vector.tensor_tensor(p3, x3, w3, OP.mult)
            nc.vector.tensor_reduce(dst, p3, axis=mybir.AxisListType.X, op=OP.add)
            nc.vector.tensor_scalar_add(dst, dst, b_q(q))

        col_sp = col_half(0, 2, "sp", cast_on_act=True)  # w_ss2 (N) | w_st2 (T)
        r_sp = const.tile([128, 2 * NT], F32)
        rstat(x_sp, NT, 0, r_sp[:, 0:NT], "r_ss")
        rstat(x_sp, NT, 2, r_sp[:, NT:], "r_st")

        # ---- per-unit emitters -------------------------------------------
        # spatial unit t (t=0..7): rows {16p+t} (h=0) and {16p+t+8} (h=1)
        # temporal unit t (t=0..1): rows 2048 + {4p+t} and 2048 + {4p+t+2}
        def pre_add(eng, dst, col, rst, nt, t, hs):
            """dst[:, h*W + region] = max(col[region] + r_region[slot], 0)"""
            for h in range(2):
                o = h * W
                s = t + h * hs
                eng.tensor_scalar(
                    dst[:, o : o + N], col[:, 0:N],
                    rst[:, s : s + 1], 0.0, OP.add, OP.max,
                )
                eng.tensor_scalar(
                    dst[:, o + N : o + W], col[:, N:W],
                    rst[:, nt + s : nt + s + 1], 0.0, OP.add, OP.max,
                )

        def store_unit(k, ot, base, nt, t, hs, split):
            quad = adj[base : base + 128 * nt, :]
            if split:
                for h in range(2):
                    s = t + h * hs
                    nc.sync.dma_start(
                        quad.rearrange("(p r) w -> p r w", p=128)[:, s : s + 1, :],
                        ot[:, h * W : (h + 1) * W].rearrange(
                            "p (r w) -> p r w", r=1
                        ),
                    )
            else:
                # partition p -> rows base + nt*p + t and base + nt*p + t + hs
                nc.sync.dma_start(
                    quad.rearrange("(p g r) w -> p g r w", p=128, g=2)[
                        :, :, t : t + 1, :
                    ],
                    ot[:].rearrange("p (g w) -> p g w", g=2).unsqueeze(2),
                )

        def act_unit(k, t, base, nt, col, rst, hs, pre_eng, split=False):
            tmp = tmpp.tile([128, 2 * W], F16, name=f"tmp{k}", tag="tmp")
            ot = outp.tile([128, 2 * W], F16, name=f"ot{k}", tag="ot")
            pre_add(pre_eng, tmp, col, rst, nt, t, hs)
            if split:
                for h in range(2):
                    nc.scalar.activation(
                        ot[:, h * W : (h + 1) * W],
                        tmp[:, h * W : (h + 1) * W], AF.Tanh,
                    )
            else:
                nc.scalar.activation(ot[:], tmp[:], AF.Tanh)
            store_unit(k, ot, base, nt, t, hs, split)

        # shifted pre-add: y = max(s + PG, PG) = relu(s) + PG, from row
        # stats pre-shifted by PG (rsh tile). x is recovered with one ts.
        def pre_add_sh(dst, col, rsh, nt, t, hs, h_list):
            for h in h_list:
                o = h * W
                s = t + h * hs
                nc.vector.tensor_scalar(
                    dst[:, o : o + N], col[:, 0:N],
                    rsh[:, s : s + 1], PG, OP.add, OP.max,
                )
                nc.vector.tensor_scalar(
                    dst[:, o + N : o + W], col[:, N:W],
                    rsh[:, nt + s : nt + s + 1], PG, OP.add, OP.max,
                )

        def poly_half(k, t, base, nt, col, rsh, hs, ot, h_list):
            # out = clip(x*(PC0 + PC1*u + PC2*u^2), 0, PCAP), u = (x+PG)^2
            lo = min(h_list) * W
            hi = (max(h_list) + 1) * W
            y = poly.tile([128, 2 * W], F16, name=f"py{k}", tag="py")
            x = poly.tile([128, 2 * W], F16, name=f"px{k}", tag="px")
            u = poly.tile([128, 2 * W], F16, name=f"pu{k}", tag="pu")
            t1 = poly.tile([128, 2 * W], F16, name=f"pt1_{k}", tag="pt1")
            t2 = poly.tile([128, 2 * W], F16, name=f"pt2_{k}", tag="pt2")
            pre_add_sh(y, col, rsh, nt, t, hs, h_list)
            yv, xv = y[:, lo:hi], x[:, lo:hi]
            uv, t1v, t2v = u[:, lo:hi], t1[:, lo:hi], t2[:, lo:hi]
            nc.vector.tensor_scalar(xv, yv, -PG, None, OP.add)
            nc.vector.tensor_tensor(uv, yv, yv, OP.mult)
            nc.vector.tensor_scalar(t1v, uv, PC2, PC1, OP.mult, OP.add)
            nc.vector.tensor_tensor(t2v, t1v, uv, OP.mult)
            nc.vector.tensor_scalar(t1v, t2v, PC0, None, OP.add)
            nc.vector.tensor_tensor(t2v, t1v, xv, OP.mult)
            nc.vector.tensor_scalar(ot[:, lo:hi], t2v, PCAP, 0.0, OP.min, OP.max)

        # shifted row stats for the poly units (y = relu(s) + PG trick)
        rp_sp = const.tile([128, 2 * NT], F32)
        nc.vector.tensor_scalar(rp_sp[:], r_sp[:], PG, None, OP.add)

        # ---- unit schedule ------------------------------------------------
        # ACT order: casts, S0h0 S0h1 S1 S2 S3 S5 S6 S7 T0 T1h0 T1h1.
        # DVE order: front, pre0, pre1, tm-casts, pre2, pre3, pre8, pre9,
        # pre5, pre6, pre7, poly_S4 — every relu-add lands before the poly
        # chain so neither ScalarE nor the store queue ever waits on it.
        # Store order: S0h0 S0h1 S1 S2 S3 S4 S5 S6 S7 T0 T1 (completion
        # order; S4 from the poly finishes ~40us, right in its slot).
        act_unit(0, 0, 0, NT, col_sp, r_sp, NT // 2, nc.vector, split=True)
        pre1 = tmpp.tile([128, 2 * W], F16, name="tmp1", tag="tmp")
        pre_add(nc.vector, pre1, col_sp, r_sp, NT, 1, NT // 2)

        col_tm = col_half(3, 1, "tm")    # cols: w_ts2 (N) | w_tt2 (T)
        r_tm = const.tile([128, 2 * TT], F32)
        rstat(x_tm, TT, 3, r_tm[:, 0:TT], "r_ts")
        rstat(x_tm, TT, 1, r_tm[:, TT:], "r_tt")

        # S1 (tmp already built above)
        ot1 = outp.tile([128, 2 * W], F16, name="ot1", tag="ot")
        nc.scalar.activation(ot1[:], pre1[:], AF.Tanh)
        store_unit(1, ot1, 0, NT, 1, NT // 2, split=False)

        act_unit(2, 2, 0, NT, col_sp, r_sp, NT // 2, nc.vector)
        act_unit(3, 3, 0, NT, col_sp, r_sp, NT // 2, nc.vector)

        # S5/S6 tmps first (ACT reaches S5 at ~38us, S6 at ~43us), then
        # the poly chain (S4 store slots between S3's and S5's at ~40us),
        # then the tail tmps, which DVE still delivers well ahead of ACT.
        tmp5 = tmpp.tile([128, 2 * W], F16, name="tmp5", tag="tmp")
        pre_add(nc.vector, tmp5, col_sp, r_sp, NT, 5, NT // 2)
        tmp6 = tmpp.tile([128, 2 * W], F16, name="tmp6", tag="tmp")
        pre_add(nc.vector, tmp6, col_sp, r_sp, NT, 6, NT // 2)

        ot4 = pout.tile([128, 2 * W], F16, name="ot4", tag="ot4")
        poly_half(4, 4, 0, NT, col_sp, rp_sp, NT // 2, ot4, [0, 1])
        store_unit(4, ot4, 0, NT, 4, NT // 2, split=False)

        tmp7 = tmpp.tile([128, 2 * W], F16, name="tmp7", tag="tmp")
        pre_add(nc.vector, tmp7, col_sp, r_sp, NT, 7, NT // 2)
        tmp8 = tmpp.tile([128, 2 * W], F16, name="tmp8", tag="tmp")
        pre_add(nc.vector, tmp8, col_tm, r_tm, TT, 0, TT // 2)
        tmp9 = tmpp.tile([128, 2 * W], F16, name="tmp9", tag="tmp")
        pre_add(nc.vector, tmp9, col_tm, r_tm, TT, 1, TT // 2)

        for k, tmpk, t, base, nt in (
            (5, tmp5, 5, 0, NT), (6, tmp6, 6, 0, NT), (7, tmp7, 7, 0, NT),
            (8, tmp8, 0, N, TT),
        ):
            otk = outp.tile([128, 2 * W], F16, name=f"ot{k}", tag="ot")
            nc.scalar.activation(otk[:], tmpk[:], AF.Tanh)
            store_unit(k, otk, base, nt, t, nt // 2, split=False)

        ot9 = outp.tile([128, 2 * W], F16, name="ot9", tag="ot")
        for h in range(2):
            nc.scalar.activation(
                ot9[:, h * W : (h + 1) * W], tmp9[:, h * W : (h + 1) * W],
                AF.Tanh,
            )
        store_unit(9, ot9, N, TT, 1, TT // 2, split=True)

def build_nc(num_devices=8):
    nc = bacc.Bacc(
        "TRN2",
        target_bir_lowering=False,
        debug=False,
        enable_asserts=True,
        num_devices=num_devices,
    )
    sp = nc.dram_tensor("spatial_nodes", (N, D), F32, kind="ExternalInput").ap()
    tm = nc.dram_tensor("temporal_nodes", (T, D), F32, kind="ExternalInput").ap()
    wp = nc.dram_tensor("wpack", (260,), F32, kind="ExternalInput").ap()
    adj = nc.dram_tensor("adj", (W, W), F16, kind="ExternalOutput").ap()

    with tile.TileContext(nc) as tc:
        _emit(tc, sp, tm, wp, adj)
    nc.compile()
    return nc


def make_in_maps(inputs):
    wpack = np.concatenate(
        [np.asarray(inputs[f"w_{nm}"], np.float32).reshape(-1) for nm in QUADS]
        + [np.asarray(inputs[f"b_{nm}"], np.float32).reshape(-1) for nm in QUADS]
    )
    in_maps = []
    for b in range(B):
        m = {
            "spatial_nodes": np.ascontiguousarray(inputs["spatial_nodes"][b], np.float32),
            "temporal_nodes": np.ascontiguousarray(inputs["temporal_nodes"][b], np.float32),
            "wpack": wpack,
        }
        in_maps.append(m)
    return in_maps


_NC = {}


def run(inputs, trace=False, trace_cores=None):
    if 8 not in _NC:
        _NC[8] = build_nc(8)
    res = run_bass_kernel_spmd(
        _NC[8], make_in_maps(inputs), core_ids=list(range(B)), trace=trace,
        trace_cores=trace_cores,
    )
    out = np.stack(
        [res.results[i]["adj"].astype(np.float32) for i in range(B)], axis=0
    )
    return out, res


def kernel(**inputs) -> np.ndarray:
    out, _ = run(inputs, trace=False)
    return out
